# revision 60
# baseline (speedup 1.0000x reference)
"""CARAFE (content-aware reassembly of features) Trainium2 Bass kernel.

Problem (hardcoded shapes):
  x       [8, 128, 64, 64] f32
  comp_w  [64, 128, 1, 1]   1x1 conv -> BN(train stats) -> SiLU
  enc_w   [100, 64, 3, 3]   3x3 conv -> BN(train stats)
  pixel_shuffle(2) -> softmax over 25 taps -> weighted 5x5 (dilation 2)
  reassembly of nearest-upsampled x. Output [8, 128, 128, 128] f32.

Sharding: data-parallel over batch, 1 image per core on 8 cores.
BN batch stats are made exact with two tiny AllGathers (sum & sumsq).

Key layout trick: with output pixel (y,x) = (2i+di, 2j+dj) and tap (dy,dx),
the reassembly source is x[c, i+dy-2, j+dx-2] -- independent of (di,dj).
So everything runs at low resolution with shifted views of a zero-padded x;
the pixel-shuffle and nearest-upsample are folded into access patterns.

Engine balance for the 25-tap weighted sum (per 8-row chunk):
  - softmax weights wsm (bf16) staged to DRAM once per chunk, then the
    mostly-idle DMA engines broadcast each tap's 4 weight rows to all 128
    partitions (stride-0 source AP): taps DMA_TAPS.
  - PE broadcasts the remaining taps' rows via one-hot matmuls (PSUM f32),
    Act casts them to bf16 SBUF: taps CAST_TAPS.
  - products run on DVE (bf16 2x mode) for most taps and on gpsimd
    (Pool queue, standard-library tensor_tensor) for POOL_TAPS.
  - the 25-tap sum is PE eye-matmul PSUM accumulation; a few DVE pair-adds
    (MERGES) trim the PE accumulate count.
Softmax/exp for chunk c+1 is pipelined with reassembly of chunk c.
"""

import sys

import numpy as np

sys.path.insert(0, "/opt/trn_rl_repo")

P = 128          # partitions / input channels
MID = 64         # compressed channels
NENC = 100       # encoder output channels = 25 taps * 4 subpixels
H = W = 64
PX = H * W       # 4096 low-res pixels per image
HP = H + 4       # zero-padded (pad=2) low-res frame for 5x5 dil-2 taps
H1 = H + 2       # zero-padded (pad=1) frame for the 3x3 conv
HM = 2 * H       # 128 upsampled
OUT = HM * HM    # 16384 output pixels per image
NB = 8           # batch / cores
NSTAT = NB * PX  # BN normalization count (N*H*W)
EPS = 1e-5
CHUNK = 512      # free-dim chunk = 8 low-res rows
NCHUNK = PX // CHUNK

# ---- per-tap engine assignment for the reassembly ----
# 25 taps: products on Pool (gpsimd tensor_tensor) for N_POOL taps, DVE for
# the rest.  Weight broadcast: PE one-hot matmul + Act cast for N_CAST taps
# (all DVE-product), DMA stride-0 broadcast for the rest.
N_POOL = 6   # products on gpsimd
N_CAST = 8   # weights broadcast via PE+Act cast (products on DVE)
N_MERGE = 3  # DVE pair-adds replacing PE accumulates

_CACHE = {}


def _tap_plan():
    """Returns (prod_engine, bcast_kind) per tap index 0..24.

    Pool-product taps and DMA-broadcast taps interleaved so every engine has
    work throughout the chunk.
    """
    prod = ["DVE"] * 25
    # spread Pool taps across the chunk
    pool_set = {0, 4, 9, 13, 17, 21} if N_POOL == 6 else set(
        round(i * 25 / max(N_POOL, 1)) for i in range(N_POOL))
    for k in list(pool_set)[:N_POOL]:
        prod[k] = "POOL"
    # CAST taps: DVE-product taps, spread out
    dve_taps = [k for k in range(25) if prod[k] == "DVE"]
    cast_set = set(dve_taps[i] for i in range(0, len(dve_taps), 2)[:N_CAST]) if False else set()
    # pick every other DVE tap until N_CAST
    cast = []
    for i, k in enumerate(dve_taps):
        if len(cast) < N_CAST and i % 2 == 1:
            cast.append(k)
    for i, k in enumerate(dve_taps):
        if len(cast) < N_CAST and i % 2 == 0:
            cast.append(k)
    bc = ["DMA"] * 25
    for k in cast:
        bc[k] = "CAST"
    return prod, bc


def _build_program():
    import concourse.bass as bass
    import concourse.mybir as mybir
    import concourse.tile as tile
    from concourse import bacc

    fp32 = mybir.dt.float32
    bf16 = mybir.dt.bfloat16
    Alu = mybir.AluOpType
    Act = mybir.ActivationFunctionType

    PROD, BC = _tap_plan()

    nc = bacc.Bacc(None, num_devices=NB)

    with tile.TileContext(nc) as tc:
        with tc.tile_pool(name="dram", bufs=1, space="DRAM") as dram:
            # I/O
            x_d = dram.tile([P, PX], bf16, kind="ExternalInput", name="x", uniquify=False)
            w1t_d = dram.tile([P, MID], bf16, kind="ExternalInput", name="w1t", uniquify=False)
            w2t_d = dram.tile([MID, 9 * NENC], bf16, kind="ExternalInput", name="w2t", uniquify=False)
            g1_d = dram.tile([MID, 2], fp32, kind="ExternalInput", name="g1b1", uniquify=False)
            g2_d = dram.tile([NENC, 2], fp32, kind="ExternalInput", name="g2b2", uniquify=False)
            sel4_d = dram.tile([NENC, 4], bf16, kind="ExternalInput", name="sel4", uniquify=False)
            sel100_d = dram.tile([4, NENC], bf16, kind="ExternalInput", name="sel100", uniquify=False)
            eye100_d = dram.tile([NENC, NENC], bf16, kind="ExternalInput", name="eye100", uniquify=False)
            eye128_d = dram.tile([P, P], bf16, kind="ExternalInput", name="eye128", uniquify=False)
            out_d = dram.tile([P, OUT], bf16, kind="ExternalOutput", name="out", uniquify=False)
            # collective bounce buffers (internal DRAM)
            ar1_in = dram.tile([2, MID], fp32, name="ar1_in")
            ar1_out = dram.tile([16, MID], fp32, name="ar1_out")
            ar2_in = dram.tile([2, NENC], fp32, name="ar2_in")
            ar2_out = dram.tile([16, NENC], fp32, name="ar2_out")
            # per-chunk weight staging for DMA broadcasts ([ch, p] row-major,
            # so taps' 4 subpixel rows are contiguous 4*CHUNK runs)
            t0_d = [dram.tile([NENC, CHUNK], bf16, name=f"t0_{c}") for c in range(NCHUNK)]

            with (
                tc.tile_pool(name="const", bufs=1) as const,
                tc.tile_pool(name="big", bufs=1) as big,
                tc.tile_pool(name="small", bufs=1) as small,
                tc.tile_pool(name="scratch", bufs=3) as scratch,
                tc.tile_pool(name="wbd", bufs=8) as wbdp,      # DMA-broadcast weights (DVE taps)
                tc.tile_pool(name="wbdP", bufs=4) as wbdPp,    # DMA-broadcast weights (Pool taps)
                tc.tile_pool(name="wbs", bufs=3) as wbsp,      # Act-cast weights
                tc.tile_pool(name="tmp", bufs=7) as tmp,       # DVE product tiles
                tc.tile_pool(name="tmpP", bufs=9) as tmpP,     # Pool product tiles (1 chunk ahead)
                tc.tile_pool(name="stg", bufs=2) as stgp,
                tc.tile_pool(name="ps", bufs=2, space="PSUM") as ps,
            ):
                # ---- load constants ----
                w1t = const.tile([P, MID], bf16)
                nc.sync.dma_start(w1t[:], w1t_d[:])
                w2t = const.tile([MID, 9 * NENC], bf16)
                nc.sync.dma_start(w2t[:], w2t_d[:])
                g1b1 = const.tile([MID, 2], fp32)
                nc.sync.dma_start(g1b1[:], g1_d[:])
                g2b2 = const.tile([NENC, 2], fp32)
                nc.sync.dma_start(g2b2[:], g2_d[:])
                sel4 = const.tile([NENC, 4], bf16)
                nc.sync.dma_start(sel4[:], sel4_d[:])
                sel100 = const.tile([4, NENC], bf16)
                nc.sync.dma_start(sel100[:], sel100_d[:])
                eye100 = const.tile([NENC, NENC], bf16)
                nc.sync.dma_start(eye100[:], eye100_d[:])
                eye128 = const.tile([P, P], bf16)
                nc.sync.dma_start(eye128[:], eye128_d[:])

                # ---- padded x (bf16) ----
                xpad = big.tile([P, HP, HP], bf16)
                nc.vector.memset(xpad[:, 0:2, :], 0.0)
                nc.vector.memset(xpad[:, 2 + H :, :], 0.0)
                nc.vector.memset(xpad[:, 2 : 2 + H, 0:2], 0.0)
                nc.vector.memset(xpad[:, 2 : 2 + H, 2 + W :], 0.0)
                xr = x_d[:].rearrange("p (h w) -> p h w", h=H)
                for hh in range(4):
                    nc.sync.dma_start(
                        xpad[:, 2 + 16 * hh : 2 + 16 * (hh + 1), 2 : 2 + W],
                        xr[:, 16 * hh : 16 * (hh + 1), :],
                    )

                # ---- conv1 (1x1, 128->64) + stats ----
                y1 = big.tile([MID, PX], bf16, tag="ybuf", name="y1")
                s1c = small.tile([MID, NCHUNK], fp32)
                ss1c = small.tile([MID, NCHUNK], fp32)
                for c in range(NCHUNK):
                    r0 = c * 8
                    pt = ps.tile(
                        [P, CHUNK], fp32, tag=("wps" if c % 2 == 0 else "sm"),
                        bufs=2, name="pt1",
                    )[:MID, :CHUNK]
                    nc.tensor.matmul(
                        pt[:], w1t[:], xpad[:, 2 + r0 : 10 + r0, 2 : 2 + W], start=True, stop=True
                    )
                    nc.scalar.activation(
                        y1[:, c * CHUNK : (c + 1) * CHUNK], pt[:], Act.Copy,
                        accum_out=s1c[:, c : c + 1],
                    )
                    sq = scratch.tile([MID, CHUNK], bf16, tag="sq1")
                    ysl = y1[:, c * CHUNK : (c + 1) * CHUNK]
                    nc.vector.tensor_tensor(sq[:], ysl, ysl, Alu.mult)
                    nc.vector.tensor_reduce(
                        ss1c[:, c : c + 1], sq[:], mybir.AxisListType.X, Alu.add
                    )

                # ---- BN1 stats allreduce ----
                st1 = small.tile([MID, 2], fp32)
                nc.vector.tensor_reduce(st1[:, 0:1], s1c[:], mybir.AxisListType.X, Alu.add)
                nc.vector.tensor_reduce(st1[:, 1:2], ss1c[:], mybir.AxisListType.X, Alu.add)
                nc.sync.dma_start(ar1_in[:], st1[:])
                # dummy Sqrt: pulls the sqrt act-table load into the AR wait
                dum1 = small.tile([1, 1], fp32, tag="dum1")
                nc.scalar.activation(dum1[:], g1b1[0:1, 0:1], Act.Sqrt)
                # AllGather + local sum: the cost model charges AllReduce a
                # 1.875x multiplier on its fixed latency; AllGather avoids it.
                nc.gpsimd.collective_compute(
                    "AllGather", Alu.bypass, replica_groups=[list(range(NB))],
                    ins=[ar1_in[:]], outs=[ar1_out[:]],
                )
                g1sb = small.tile([MID, 2 * NB], fp32)
                agt = ar1_out[:]
                # DRAM blob is [core][ch][j]; read as (ch, core, j)
                nc.sync.dma_start(
                    g1sb[:],
                    bass.AP(agt.tensor, agt.offset, [[2, MID], [2 * MID, NB], [1, 2]]),
                )
                st1r = small.tile([MID, 2], fp32)
                nc.vector.tensor_reduce(
                    st1r[:, 0:1],
                    bass.AP(g1sb.tensor, g1sb.offset, [g1sb.ap[0], [2, NB]]),
                    mybir.AxisListType.X, Alu.add,
                )
                nc.vector.tensor_reduce(
                    st1r[:, 1:2],
                    bass.AP(g1sb.tensor, g1sb.offset + 1, [g1sb.ap[0], [2, NB]]),
                    mybir.AxisListType.X, Alu.add,
                )

                def bn_coeffs(pool, stats, gb, nchan, tag):
                    # stats [C,2] (sum, sumsq) -> scale/bias [C,1] each
                    m = pool.tile([nchan, 4], fp32, tag=tag)
                    nc.vector.tensor_scalar_mul(m[:, 0:1], stats[:, 0:1], 1.0 / NSTAT)
                    nc.vector.tensor_scalar_mul(m[:, 1:2], stats[:, 1:2], 1.0 / NSTAT)
                    nc.vector.tensor_tensor(m[:, 2:3], m[:, 0:1], m[:, 0:1], Alu.mult)
                    nc.vector.tensor_tensor(m[:, 3:4], m[:, 1:2], m[:, 2:3], Alu.subtract)
                    epst = pool.tile([nchan, 1], fp32, tag=tag + "e")
                    nc.vector.memset(epst[:], EPS)
                    std = pool.tile([nchan, 1], fp32, tag=tag + "s")
                    nc.scalar.activation(std[:], m[:, 3:4], Act.Sqrt, bias=epst[:])
                    inv = pool.tile([nchan, 1], fp32, tag=tag + "i")
                    nc.vector.reciprocal(inv[:], std[:])
                    sc = pool.tile([nchan, 2], fp32, tag=tag + "c")
                    # scale = gamma * inv ; bias = beta - mean*scale
                    nc.vector.tensor_tensor(sc[:, 0:1], gb[:, 0:1], inv[:], Alu.mult)
                    tmpm = pool.tile([nchan, 1], fp32, tag=tag + "m")
                    nc.vector.tensor_tensor(tmpm[:], m[:, 0:1], sc[:, 0:1], Alu.mult)
                    nc.vector.tensor_tensor(sc[:, 1:2], gb[:, 1:2], tmpm[:], Alu.subtract)
                    return sc

                sc1 = bn_coeffs(small, st1r, g1b1, MID, "bn1")

                # ---- BN1 + SiLU into padded t1 (bf16) ----
                t1pad = big.tile([MID, H1, H1], bf16)
                nc.vector.memset(t1pad[:, 0:1, :], 0.0)
                nc.vector.memset(t1pad[:, 1 + H :, :], 0.0)
                nc.vector.memset(t1pad[:, 1 : 1 + H, 0:1], 0.0)
                nc.vector.memset(t1pad[:, 1 : 1 + H, 1 + W :], 0.0)
                for c in range(NCHUNK):
                    r0 = c * 8
                    nc.scalar.activation(
                        t1pad[:, 1 + r0 : 9 + r0, 1 : 1 + W],
                        y1[:, c * CHUNK : (c + 1) * CHUNK],
                        Act.Silu, bias=sc1[:, 1:2], scale=sc1[:, 0:1],
                    )

                # ---- conv2 (3x3, 64->100) + stats ----
                y2 = big.tile([NENC, PX], bf16)
                s2c = small.tile([NENC, NCHUNK], fp32)
                ss2c = small.tile([NENC, NCHUNK], fp32)
                for c in range(NCHUNK):
                    r0 = c * 8
                    pt = ps.tile(
                        [P, CHUNK], fp32, tag=("wps" if c % 2 == 0 else "sm"),
                        bufs=2, name="pt2",
                    )[:NENC, :CHUNK]
                    for tap in range(9):
                        dy, dx = tap // 3, tap % 3
                        nc.tensor.matmul(
                            pt[:],
                            w2t[:, tap * NENC : (tap + 1) * NENC],
                            t1pad[:, r0 + dy : r0 + dy + 8, dx : dx + W],
                            start=(tap == 0), stop=(tap == 8),
                        )
                    nc.scalar.activation(
                        y2[:, c * CHUNK : (c + 1) * CHUNK], pt[:], Act.Copy,
                        accum_out=s2c[:, c : c + 1],
                    )
                    sq = scratch.tile([NENC, CHUNK], bf16, tag="sq2")
                    ysl2 = y2[:, c * CHUNK : (c + 1) * CHUNK]
                    nc.vector.tensor_tensor(sq[:], ysl2, ysl2, Alu.mult)
                    nc.vector.tensor_reduce(
                        ss2c[:, c : c + 1], sq[:], mybir.AxisListType.X, Alu.add
                    )

                # ---- BN2 stats allreduce ----
                st2 = small.tile([NENC, 2], fp32)
                nc.vector.tensor_reduce(st2[:, 0:1], s2c[:], mybir.AxisListType.X, Alu.add)
                nc.vector.tensor_reduce(st2[:, 1:2], ss2c[:], mybir.AxisListType.X, Alu.add)
                nc.sync.dma_start(ar2_in[:], st2[:])
                # dummy Sqrt: pulls the sqrt act-table load into the AR wait
                dum2 = small.tile([1, 1], fp32, tag="dum2")
                nc.scalar.activation(dum2[:], g2b2[0:1, 0:1], Act.Sqrt)
                nc.gpsimd.collective_compute(
                    "AllGather", Alu.bypass, replica_groups=[list(range(NB))],
                    ins=[ar2_in[:]], outs=[ar2_out[:]],
                )
                g2sb = small.tile([NENC, 2 * NB], fp32)
                agt2 = ar2_out[:]
                nc.sync.dma_start(
                    g2sb[:],
                    bass.AP(agt2.tensor, agt2.offset, [[2, NENC], [2 * NENC, NB], [1, 2]]),
                )
                st2r = small.tile([NENC, 2], fp32)
                nc.vector.tensor_reduce(
                    st2r[:, 0:1],
                    bass.AP(g2sb.tensor, g2sb.offset, [g2sb.ap[0], [2, NB]]),
                    mybir.AxisListType.X, Alu.add,
                )
                nc.vector.tensor_reduce(
                    st2r[:, 1:2],
                    bass.AP(g2sb.tensor, g2sb.offset + 1, [g2sb.ap[0], [2, NB]]),
                    mybir.AxisListType.X, Alu.add,
                )
                sc2 = bn_coeffs(small, st2r, g2b2, NENC, "bn2")

                # ---- per-chunk softmax (pipelined ahead of reassembly) ----
                esm = big.tile([NENC, PX], bf16, name="esm")
                wsm = big.tile([NENC, PX], bf16, name="wsm")
                r4 = big.tile([4, PX], bf16)

                def softmax_chunk(c):
                    sl = slice(c * CHUNK, (c + 1) * CHUNK)
                    # BN output is ~N(0,1): exp without max-subtraction is
                    # safe in f32.
                    nc.scalar.activation(
                        esm[:, sl], y2[:, sl], Act.Exp, bias=sc2[:, 1:2], scale=sc2[:, 0:1]
                    )
                    pd = ps.tile([P, CHUNK], fp32, tag="sm", bufs=2, name="pd")[:4, :CHUNK]
                    nc.tensor.matmul(pd[:], sel4[:], esm[:, sl], start=True, stop=True)
                    with nc.allow_low_precision("softmax denominators: bf16 ample for 2e-2 tolerance"):
                        nc.vector.reciprocal(r4[:, sl], pd[:])
                    pr = ps.tile([P, CHUNK], fp32, tag="sm", bufs=2, name="pr")[:NENC, :CHUNK]
                    nc.tensor.matmul(pr[:], sel100[:], r4[:, sl], start=True, stop=True)
                    nc.vector.tensor_tensor(wsm[:, sl], esm[:, sl], pr[:], Alu.mult)
                    # stage weights to DRAM for the DMA broadcasts
                    nc.sync.dma_start(t0_d[c][:], wsm[:, sl])

                # ---- reassembly ----
                def xview(k, r0, rep):
                    dy, dx = k // 5, k % 5
                    xv = xpad[:, r0 + dy : r0 + dy + 8, dx : dx + W]
                    return bass.AP(
                        xv.tensor, xv.offset, [xv.ap[0], [0, rep]] + list(xv.ap[1:])
                    )

                pool_taps_fixed = [k for k in range(25) if PROD[k] == "POOL"]

                def bcast_dma_for(c, k, pool):
                    wb = pool.tile([P, 4 * CHUNK], bf16, tag="wbd", name="wbd")
                    src = bass.AP(
                        t0_d[c].tensor,
                        t0_d[c].offset + 4 * k * CHUNK,
                        [[0, P], [1, 4 * CHUNK]],
                    )
                    nc.sync.dma_start(wb[:], src)
                    return wb

                def pool_product(c, k, wb):
                    tm = tmpP.tile([P, 4 * CHUNK], bf16, tag="tmP", name="tmP")
                    nc.gpsimd.tensor_tensor(tm[:], wb[:], xview(k, c * 8, 4), Alu.mult)
                    return tm

                def pool_chunk(c):
                    # emit all Pool-tap work for chunk c (runs one chunk ahead)
                    res = {}
                    wbs = [(k, bcast_dma_for(c, k, wbdPp)) for k in pool_taps_fixed]
                    for k, wb in wbs:
                        res[k] = pool_product(c, k, wb)
                    return res

                softmax_chunk(0)
                pool_tm = pool_chunk(0)
                softmax_chunk(1)

                for c in range(NCHUNK):
                    r0 = c * 8
                    col = slice(c * CHUNK, (c + 1) * CHUNK)
                    # 4 independent 1-bank accumulators: the next chunk can
                    # reuse bank s as soon as its stage copy drains it.
                    acc_ps = [
                        ps.tile([P, CHUNK], fp32, tag=f"acc{s}", bufs=1, name=f"acc{s}")
                        for s in range(4)
                    ]

                    # broadcast producers -------------------------------------
                    def bcast_cast(k):
                        wbS = wbsp.tile([P, 4 * CHUNK], bf16, tag="wbs", name="wbS")
                        for h in range(4):
                            wps = ps.tile(
                                [P, CHUNK], fp32, tag=("wps" if h % 2 == 0 else "sm"),
                                bufs=2, name="wps",
                            )
                            ch = 4 * k + h
                            onehot = eye100[:, ch : ch + 1].to_broadcast((NENC, P))
                            nc.tensor.matmul(
                                wps[:], onehot, wsm[:, col], start=True, stop=True
                            )
                            nc.scalar.activation(
                                wbS[:, h * CHUNK : (h + 1) * CHUNK], wps[:], Act.Copy
                            )
                        return wbS

                    def dve_product(k, wb):
                        tm = tmp.tile([P, 4 * CHUNK], bf16, tag="tm", name="tm")
                        nc.vector.tensor_tensor(tm[:], wb[:], xview(k, r0, 4), Alu.mult)
                        return tm

                    def acc(tm, first, last):
                        for s in range(4):
                            nc.tensor.matmul(
                                acc_ps[s][:], eye128[:],
                                tm[:, s * CHUNK : (s + 1) * CHUNK],
                                start=first, stop=last, skip_group_check=True,
                            )

                    # accumulation order: pool taps (precomputed last chunk)
                    # interleaved with DVE taps so acc can start immediately.
                    dve_taps = [k for k in range(25) if PROD[k] != "POOL"]
                    if c < 2:
                        # startup: CAST weights (PE+Act) are ready ~4us before
                        # the DRAM-staged DMA broadcasts; pool products land
                        # late (DMA-fed) so their accumulates go last.
                        cast_first = [k for k in dve_taps if BC[k] == "CAST"]
                        dma_rest = [k for k in dve_taps if BC[k] != "CAST"]
                        order = cast_first + dma_rest + list(pool_taps_fixed)
                    else:
                        # front-load 3 ready pool tiles, spread the rest
                        pool_pos = {0, 1, 2, 3, 14, 20}
                        order = []
                        li, pi = 0, 0
                        for i in range(25):
                            if pi < len(pool_taps_fixed) and i in pool_pos:
                                order.append(pool_taps_fixed[pi]); pi += 1
                            else:
                                order.append(dve_taps[li]); li += 1
                        order += pool_taps_fixed[pi:] + dve_taps[li:]

                    def prep(k):
                        if BC[k] == "CAST":
                            return bcast_cast(k)
                        return bcast_dma_for(c, k, wbdp)

                    merge_pts = set()
                    if N_MERGE > 0:
                        step = max(1, 25 // (N_MERGE + 1))
                        merge_pts = {step * (i + 1) for i in range(N_MERGE)}
                    dve_order = [k for k in order if PROD[k] != "POOL"]
                    pres = {}
                    AHEAD = 7
                    for i in range(min(AHEAD, len(dve_order))):
                        pres[dve_order[i]] = prep(dve_order[i])
                    npre = min(AHEAD, len(dve_order))
                    # next chunk's pool work interleaves with this chunk
                    nxt_pool = {}
                    pool_emit = {4: "wb", 8: "prod"} if False else None
                    nxt_wbs = []
                    pending = None
                    started = False
                    di = 0  # index into dve_order of next prep to emit
                    di = npre
                    for i in range(25):
                        k = order[i]
                        if PROD[k] == "POOL":
                            tm = pool_tm.pop(k)
                            # interleave next chunk's pool broadcasts/products
                            if c + 1 < NCHUNK:
                                j = pool_taps_fixed.index(k)
                                kj = pool_taps_fixed[j]
                                nxt_wbs.append((kj, bcast_dma_for(c + 1, kj, wbdPp)))
                        else:
                            if di < len(dve_order):
                                pres[dve_order[di]] = prep(dve_order[di])
                                di += 1
                            tm = dve_product(k, pres.pop(k))
                        if i in merge_pts and pending is not None:
                            tm2 = tmp.tile([P, 4 * CHUNK], bf16, tag="tm", name="tmm")
                            nc.vector.tensor_tensor(tm2[:], tm[:], pending[:], Alu.add)
                            pending = tm2
                        else:
                            if pending is not None:
                                acc(pending, first=not started, last=False)
                                started = True
                            pending = tm
                        # emit next chunk's pool products spread through chunk
                        if c + 1 < NCHUNK and nxt_wbs and i % 4 == 2:
                            kj, wbj = nxt_wbs.pop(0)
                            nxt_pool[kj] = pool_product(c + 1, kj, wbj)
                    acc(pending, first=not started, last=True)
                    if c + 1 < NCHUNK:
                        for kj, wbj in nxt_wbs:
                            nxt_pool[kj] = pool_product(c + 1, kj, wbj)
                        pool_tm = nxt_pool

                    # de-interleave (s=(di,dj), i, j) -> out rows (2i+di), cols (2j+dj)
                    # (emitted before the pipelined softmax so the stage copies
                    # sit early in the Act queue and release acc_ps quickly)
                    stage = stgp.tile([P, 4 * CHUNK], bf16, tag="stg", name="stage")
                    for s in range(4):
                        di_, dj_ = s // 2, s % 2
                        stg_v = bass.AP(
                            stage.tensor, stage.offset + di_ * 128 + dj_,
                            [stage.ap[0], [256, 8], [2, 64]],
                        )
                        acc_v = bass.AP(
                            acc_ps[s].tensor, acc_ps[s].offset,
                            [acc_ps[s].ap[0], [64, 8], [1, 64]],
                        )
                        nc.scalar.activation(stg_v, acc_v, Act.Copy)
                    nc.sync.dma_start(
                        out_d[:, c * 4 * CHUNK : (c + 1) * 4 * CHUNK], stage[:]
                    )

                    # pipeline the next softmax chunk while PE accumulates
                    if c + 2 < NCHUNK:
                        softmax_chunk(c + 2)

    nc.compile()
    return nc


def _prep_shared(comp_w, comp_g, comp_b, enc_w, enc_g, enc_b):
    import ml_dtypes

    bf = ml_dtypes.bfloat16
    w1t = np.ascontiguousarray(comp_w.reshape(MID, P).T).astype(bf)      # [128, 64]
    # w2t[tap] = enc_w[:, :, dy, dx].T  -> [64, 100] per tap, taps flattened
    w2t = np.ascontiguousarray(
        enc_w.transpose(2, 3, 1, 0).reshape(9, MID, NENC).transpose(1, 0, 2).reshape(MID, 9 * NENC)
    ).astype(bf)
    g1b1 = np.stack([comp_g, comp_b], axis=1).astype(np.float32)    # [64, 2]
    g2b2 = np.stack([enc_g, enc_b], axis=1).astype(np.float32)      # [100, 2]
    ch = np.arange(NENC)
    sel4 = (ch[:, None] % 4 == np.arange(4)[None, :]).astype(bf)   # [100, 4]
    sel100 = np.ascontiguousarray(sel4.T)                           # [4, 100]
    eye100 = np.eye(NENC, dtype=np.float32).astype(bf)
    eye128 = np.eye(P, dtype=np.float32).astype(bf)
    return dict(w1t=w1t, w2t=w2t, g1b1=g1b1, g2b2=g2b2, sel4=sel4, sel100=sel100, eye100=eye100, eye128=eye128)


def kernel(x, comp_w, comp_g, comp_b, enc_w, enc_g, enc_b):
    import ml_dtypes

    from concourse.bass_utils import run_bass_kernel_spmd

    x = np.asarray(x, np.float32)
    shared = _prep_shared(
        np.asarray(comp_w, np.float32), np.asarray(comp_g, np.float32),
        np.asarray(comp_b, np.float32), np.asarray(enc_w, np.float32),
        np.asarray(enc_g, np.float32), np.asarray(enc_b, np.float32),
    )
    if "nc" not in _CACHE:
        _CACHE["nc"] = _build_program()
    nc = _CACHE["nc"]

    in_maps = []
    for i in range(NB):
        m = dict(shared)
        m["x"] = np.ascontiguousarray(x[i].reshape(P, PX)).astype(ml_dtypes.bfloat16)
        in_maps.append(m)

    res = run_bass_kernel_spmd(nc, in_maps, list(range(NB)))
    out = np.stack([
        res.results[i]["out"].astype(np.float32).reshape(P, HM, HM) for i in range(NB)
    ])
    return out.astype(np.float32)


# revision 61
# speedup vs baseline: 1.0084x; 1.0084x over previous
"""CARAFE (content-aware reassembly of features) Trainium2 Bass kernel.

Problem (hardcoded shapes):
  x       [8, 128, 64, 64] f32
  comp_w  [64, 128, 1, 1]   1x1 conv -> BN(train stats) -> SiLU
  enc_w   [100, 64, 3, 3]   3x3 conv -> BN(train stats)
  pixel_shuffle(2) -> softmax over 25 taps -> weighted 5x5 (dilation 2)
  reassembly of nearest-upsampled x. Output [8, 128, 128, 128] f32.

Sharding: data-parallel over batch, 1 image per core on 8 cores.
BN batch stats are made exact with two tiny AllGathers (sum & sumsq).

Key layout trick: with output pixel (y,x) = (2i+di, 2j+dj) and tap (dy,dx),
the reassembly source is x[c, i+dy-2, j+dx-2] -- independent of (di,dj).
So everything runs at low resolution with shifted views of a zero-padded x;
the pixel-shuffle and nearest-upsample are folded into access patterns.

Engine balance for the 25-tap weighted sum (per 8-row chunk):
  - softmax weights wsm (bf16) staged to DRAM once per chunk, then the
    mostly-idle DMA engines broadcast each tap's 4 weight rows to all 128
    partitions (stride-0 source AP): taps DMA_TAPS.
  - PE broadcasts the remaining taps' rows via one-hot matmuls (PSUM f32),
    Act casts them to bf16 SBUF: taps CAST_TAPS.
  - products run on DVE (bf16 2x mode) for most taps and on gpsimd
    (Pool queue, standard-library tensor_tensor) for POOL_TAPS.
  - the 25-tap sum is PE eye-matmul PSUM accumulation; a few DVE pair-adds
    (MERGES) trim the PE accumulate count.
Softmax/exp for chunk c+1 is pipelined with reassembly of chunk c.
"""

import sys

import numpy as np

sys.path.insert(0, "/opt/trn_rl_repo")

P = 128          # partitions / input channels
MID = 64         # compressed channels
NENC = 100       # encoder output channels = 25 taps * 4 subpixels
H = W = 64
PX = H * W       # 4096 low-res pixels per image
HP = H + 4       # zero-padded (pad=2) low-res frame for 5x5 dil-2 taps
H1 = H + 2       # zero-padded (pad=1) frame for the 3x3 conv
HM = 2 * H       # 128 upsampled
OUT = HM * HM    # 16384 output pixels per image
NB = 8           # batch / cores
NSTAT = NB * PX  # BN normalization count (N*H*W)
EPS = 1e-5
CHUNK = 512      # free-dim chunk = 8 low-res rows
NCHUNK = PX // CHUNK

# ---- per-tap engine assignment for the reassembly ----
# 25 taps: products on Pool (gpsimd tensor_tensor) for N_POOL taps, DVE for
# the rest.  Weight broadcast: PE one-hot matmul + Act cast for N_CAST taps
# (all DVE-product), DMA stride-0 broadcast for the rest.
N_POOL = 6   # products on gpsimd
N_CAST = 8   # weights broadcast via PE+Act cast (products on DVE)
N_MERGE = 3  # DVE pair-adds replacing PE accumulates

_CACHE = {}


def _tap_plan():
    """Returns (prod_engine, bcast_kind) per tap index 0..24.

    Pool-product taps and DMA-broadcast taps interleaved so every engine has
    work throughout the chunk.
    """
    prod = ["DVE"] * 25
    # spread Pool taps across the chunk
    pool_set = {0, 4, 9, 13, 17, 21} if N_POOL == 6 else set(
        round(i * 25 / max(N_POOL, 1)) for i in range(N_POOL))
    for k in list(pool_set)[:N_POOL]:
        prod[k] = "POOL"
    # CAST taps: DVE-product taps, spread out
    dve_taps = [k for k in range(25) if prod[k] == "DVE"]
    cast_set = set(dve_taps[i] for i in range(0, len(dve_taps), 2)[:N_CAST]) if False else set()
    # pick every other DVE tap until N_CAST
    cast = []
    for i, k in enumerate(dve_taps):
        if len(cast) < N_CAST and i % 2 == 1:
            cast.append(k)
    for i, k in enumerate(dve_taps):
        if len(cast) < N_CAST and i % 2 == 0:
            cast.append(k)
    bc = ["DMA"] * 25
    for k in cast:
        bc[k] = "CAST"
    return prod, bc


def _build_program():
    import concourse.bass as bass
    import concourse.mybir as mybir
    import concourse.tile as tile
    from concourse import bacc

    fp32 = mybir.dt.float32
    bf16 = mybir.dt.bfloat16
    Alu = mybir.AluOpType
    Act = mybir.ActivationFunctionType

    PROD, BC = _tap_plan()

    nc = bacc.Bacc(None, num_devices=NB)

    with tile.TileContext(nc) as tc:
        with tc.tile_pool(name="dram", bufs=1, space="DRAM") as dram:
            # I/O
            x_d = dram.tile([P, PX], bf16, kind="ExternalInput", name="x", uniquify=False)
            w1t_d = dram.tile([P, MID], bf16, kind="ExternalInput", name="w1t", uniquify=False)
            w2t_d = dram.tile([MID, 9 * NENC], bf16, kind="ExternalInput", name="w2t", uniquify=False)
            g1_d = dram.tile([MID, 2], fp32, kind="ExternalInput", name="g1b1", uniquify=False)
            g2_d = dram.tile([NENC, 2], fp32, kind="ExternalInput", name="g2b2", uniquify=False)
            sel4_d = dram.tile([NENC, 4], bf16, kind="ExternalInput", name="sel4", uniquify=False)
            sel100_d = dram.tile([4, NENC], bf16, kind="ExternalInput", name="sel100", uniquify=False)
            eye100_d = dram.tile([NENC, NENC], bf16, kind="ExternalInput", name="eye100", uniquify=False)
            eye128_d = dram.tile([P, P], bf16, kind="ExternalInput", name="eye128", uniquify=False)
            out_d = dram.tile([P, OUT], bf16, kind="ExternalOutput", name="out", uniquify=False)
            # collective bounce buffers (internal DRAM)
            ar1_in = dram.tile([2, MID], fp32, name="ar1_in")
            ar1_out = dram.tile([16, MID], fp32, name="ar1_out")
            ar2_in = dram.tile([2, NENC], fp32, name="ar2_in")
            ar2_out = dram.tile([16, NENC], fp32, name="ar2_out")
            # per-chunk weight staging for DMA broadcasts ([ch, p] row-major,
            # so taps' 4 subpixel rows are contiguous 4*CHUNK runs)
            t0_d = [dram.tile([NENC, CHUNK], bf16, name=f"t0_{c}") for c in range(NCHUNK)]

            with (
                tc.tile_pool(name="const", bufs=1) as const,
                tc.tile_pool(name="big", bufs=1) as big,
                tc.tile_pool(name="small", bufs=1) as small,
                tc.tile_pool(name="scratch", bufs=3) as scratch,
                tc.tile_pool(name="wbd", bufs=8) as wbdp,      # DMA-broadcast weights (DVE taps)
                tc.tile_pool(name="wbdP", bufs=4) as wbdPp,    # DMA-broadcast weights (Pool taps)
                tc.tile_pool(name="wbs", bufs=3) as wbsp,      # Act-cast weights
                tc.tile_pool(name="tmp", bufs=7) as tmp,       # DVE product tiles
                tc.tile_pool(name="tmpP", bufs=9) as tmpP,     # Pool product tiles (1 chunk ahead)
                tc.tile_pool(name="stg", bufs=2) as stgp,
                tc.tile_pool(name="ps", bufs=2, space="PSUM") as ps,
            ):
                # ---- load constants ----
                w1t = const.tile([P, MID], bf16)
                nc.sync.dma_start(w1t[:], w1t_d[:])
                w2t = const.tile([MID, 9 * NENC], bf16)
                nc.sync.dma_start(w2t[:], w2t_d[:])
                g1b1 = const.tile([MID, 2], fp32)
                nc.sync.dma_start(g1b1[:], g1_d[:])
                g2b2 = const.tile([NENC, 2], fp32)
                nc.sync.dma_start(g2b2[:], g2_d[:])
                sel4 = const.tile([NENC, 4], bf16)
                nc.sync.dma_start(sel4[:], sel4_d[:])
                sel100 = const.tile([4, NENC], bf16)
                nc.sync.dma_start(sel100[:], sel100_d[:])
                eye100 = const.tile([NENC, NENC], bf16)
                nc.sync.dma_start(eye100[:], eye100_d[:])
                eye128 = const.tile([P, P], bf16)
                nc.sync.dma_start(eye128[:], eye128_d[:])

                # ---- padded x (bf16) ----
                xpad = big.tile([P, HP, HP], bf16)
                nc.vector.memset(xpad[:, 0:2, :], 0.0)
                nc.vector.memset(xpad[:, 2 + H :, :], 0.0)
                nc.vector.memset(xpad[:, 2 : 2 + H, 0:2], 0.0)
                nc.vector.memset(xpad[:, 2 : 2 + H, 2 + W :], 0.0)
                xr = x_d[:].rearrange("p (h w) -> p h w", h=H)
                for hh in range(4):
                    nc.sync.dma_start(
                        xpad[:, 2 + 16 * hh : 2 + 16 * (hh + 1), 2 : 2 + W],
                        xr[:, 16 * hh : 16 * (hh + 1), :],
                    )

                # ---- conv1 (1x1, 128->64) + stats ----
                y1 = big.tile([MID, PX], bf16, tag="ybuf", name="y1")
                s1c = small.tile([MID, NCHUNK], fp32)
                ss1c = small.tile([MID, NCHUNK], fp32)
                for c in range(NCHUNK):
                    r0 = c * 8
                    pt = ps.tile(
                        [P, CHUNK], fp32, tag=("wps" if c % 2 == 0 else "sm"),
                        bufs=2, name="pt1",
                    )[:MID, :CHUNK]
                    nc.tensor.matmul(
                        pt[:], w1t[:], xpad[:, 2 + r0 : 10 + r0, 2 : 2 + W], start=True, stop=True
                    )
                    nc.scalar.activation(
                        y1[:, c * CHUNK : (c + 1) * CHUNK], pt[:], Act.Copy,
                        accum_out=s1c[:, c : c + 1],
                    )
                    sq = scratch.tile([MID, CHUNK], bf16, tag="sq1")
                    ysl = y1[:, c * CHUNK : (c + 1) * CHUNK]
                    nc.vector.tensor_tensor(sq[:], ysl, ysl, Alu.mult)
                    nc.vector.tensor_reduce(
                        ss1c[:, c : c + 1], sq[:], mybir.AxisListType.X, Alu.add
                    )

                # ---- BN1 stats allreduce ----
                st1 = small.tile([MID, 2], fp32)
                nc.vector.tensor_reduce(st1[:, 0:1], s1c[:], mybir.AxisListType.X, Alu.add)
                nc.vector.tensor_reduce(st1[:, 1:2], ss1c[:], mybir.AxisListType.X, Alu.add)
                nc.sync.dma_start(ar1_in[:], st1[:])
                # dummy Sqrt: pulls the sqrt act-table load into the AR wait
                dum1 = small.tile([1, 1], fp32, tag="dum1")
                nc.scalar.activation(dum1[:], g1b1[0:1, 0:1], Act.Sqrt)
                # AllGather + local sum: the cost model charges AllReduce a
                # 1.875x multiplier on its fixed latency; AllGather avoids it.
                nc.gpsimd.collective_compute(
                    "AllGather", Alu.bypass, replica_groups=[list(range(NB))],
                    ins=[ar1_in[:]], outs=[ar1_out[:]],
                )
                g1sb = small.tile([MID, 2 * NB], fp32)
                agt = ar1_out[:]
                # DRAM blob is [core][ch][j]; read as (ch, core, j)
                nc.sync.dma_start(
                    g1sb[:],
                    bass.AP(agt.tensor, agt.offset, [[2, MID], [2 * MID, NB], [1, 2]]),
                )
                st1r = small.tile([MID, 2], fp32)
                nc.vector.tensor_reduce(
                    st1r[:, 0:1],
                    bass.AP(g1sb.tensor, g1sb.offset, [g1sb.ap[0], [2, NB]]),
                    mybir.AxisListType.X, Alu.add,
                )
                nc.vector.tensor_reduce(
                    st1r[:, 1:2],
                    bass.AP(g1sb.tensor, g1sb.offset + 1, [g1sb.ap[0], [2, NB]]),
                    mybir.AxisListType.X, Alu.add,
                )

                def bn_coeffs(pool, stats, gb, nchan, tag):
                    # stats [C,2] (sum, sumsq) -> scale/bias [C,1] each
                    m = pool.tile([nchan, 4], fp32, tag=tag)
                    nc.vector.tensor_scalar_mul(m[:, 0:1], stats[:, 0:1], 1.0 / NSTAT)
                    nc.vector.tensor_scalar_mul(m[:, 1:2], stats[:, 1:2], 1.0 / NSTAT)
                    nc.vector.tensor_tensor(m[:, 2:3], m[:, 0:1], m[:, 0:1], Alu.mult)
                    nc.vector.tensor_tensor(m[:, 3:4], m[:, 1:2], m[:, 2:3], Alu.subtract)
                    epst = pool.tile([nchan, 1], fp32, tag=tag + "e")
                    nc.vector.memset(epst[:], EPS)
                    std = pool.tile([nchan, 1], fp32, tag=tag + "s")
                    nc.scalar.activation(std[:], m[:, 3:4], Act.Sqrt, bias=epst[:])
                    inv = pool.tile([nchan, 1], fp32, tag=tag + "i")
                    nc.vector.reciprocal(inv[:], std[:])
                    sc = pool.tile([nchan, 2], fp32, tag=tag + "c")
                    # scale = gamma * inv ; bias = beta - mean*scale
                    nc.vector.tensor_tensor(sc[:, 0:1], gb[:, 0:1], inv[:], Alu.mult)
                    tmpm = pool.tile([nchan, 1], fp32, tag=tag + "m")
                    nc.vector.tensor_tensor(tmpm[:], m[:, 0:1], sc[:, 0:1], Alu.mult)
                    nc.vector.tensor_tensor(sc[:, 1:2], gb[:, 1:2], tmpm[:], Alu.subtract)
                    return sc

                sc1 = bn_coeffs(small, st1r, g1b1, MID, "bn1")

                # ---- BN1 + SiLU into padded t1 (bf16) ----
                t1pad = big.tile([MID, H1, H1], bf16)
                nc.vector.memset(t1pad[:, 0:1, :], 0.0)
                nc.vector.memset(t1pad[:, 1 + H :, :], 0.0)
                nc.vector.memset(t1pad[:, 1 : 1 + H, 0:1], 0.0)
                nc.vector.memset(t1pad[:, 1 : 1 + H, 1 + W :], 0.0)
                for c in range(NCHUNK):
                    r0 = c * 8
                    nc.scalar.activation(
                        t1pad[:, 1 + r0 : 9 + r0, 1 : 1 + W],
                        y1[:, c * CHUNK : (c + 1) * CHUNK],
                        Act.Silu, bias=sc1[:, 1:2], scale=sc1[:, 0:1],
                    )

                # ---- conv2 (3x3, 64->100) + stats ----
                y2 = big.tile([NENC, PX], bf16)
                s2c = small.tile([NENC, NCHUNK], fp32)
                ss2c = small.tile([NENC, NCHUNK], fp32)
                for c in range(NCHUNK):
                    r0 = c * 8
                    pt = ps.tile(
                        [P, CHUNK], fp32, tag=("wps" if c % 2 == 0 else "sm"),
                        bufs=2, name="pt2",
                    )[:NENC, :CHUNK]
                    for tap in range(9):
                        dy, dx = tap // 3, tap % 3
                        nc.tensor.matmul(
                            pt[:],
                            w2t[:, tap * NENC : (tap + 1) * NENC],
                            t1pad[:, r0 + dy : r0 + dy + 8, dx : dx + W],
                            start=(tap == 0), stop=(tap == 8),
                        )
                    nc.scalar.activation(
                        y2[:, c * CHUNK : (c + 1) * CHUNK], pt[:], Act.Copy,
                        accum_out=s2c[:, c : c + 1],
                    )
                    sq = scratch.tile([NENC, CHUNK], bf16, tag="sq2")
                    ysl2 = y2[:, c * CHUNK : (c + 1) * CHUNK]
                    nc.vector.tensor_tensor(sq[:], ysl2, ysl2, Alu.mult)
                    nc.vector.tensor_reduce(
                        ss2c[:, c : c + 1], sq[:], mybir.AxisListType.X, Alu.add
                    )

                # ---- BN2 stats allreduce ----
                st2 = small.tile([NENC, 2], fp32)
                nc.vector.tensor_reduce(st2[:, 0:1], s2c[:], mybir.AxisListType.X, Alu.add)
                nc.vector.tensor_reduce(st2[:, 1:2], ss2c[:], mybir.AxisListType.X, Alu.add)
                nc.sync.dma_start(ar2_in[:], st2[:])
                # dummy Sqrt: pulls the sqrt act-table load into the AR wait
                dum2 = small.tile([1, 1], fp32, tag="dum2")
                nc.scalar.activation(dum2[:], g2b2[0:1, 0:1], Act.Sqrt)
                nc.gpsimd.collective_compute(
                    "AllGather", Alu.bypass, replica_groups=[list(range(NB))],
                    ins=[ar2_in[:]], outs=[ar2_out[:]],
                )
                g2sb = small.tile([NENC, 2 * NB], fp32)
                agt2 = ar2_out[:]
                nc.sync.dma_start(
                    g2sb[:],
                    bass.AP(agt2.tensor, agt2.offset, [[2, NENC], [2 * NENC, NB], [1, 2]]),
                )
                st2r = small.tile([NENC, 2], fp32)
                nc.vector.tensor_reduce(
                    st2r[:, 0:1],
                    bass.AP(g2sb.tensor, g2sb.offset, [g2sb.ap[0], [2, NB]]),
                    mybir.AxisListType.X, Alu.add,
                )
                nc.vector.tensor_reduce(
                    st2r[:, 1:2],
                    bass.AP(g2sb.tensor, g2sb.offset + 1, [g2sb.ap[0], [2, NB]]),
                    mybir.AxisListType.X, Alu.add,
                )
                sc2 = bn_coeffs(small, st2r, g2b2, NENC, "bn2")

                # ---- per-chunk softmax (pipelined ahead of reassembly) ----
                esm = big.tile([NENC, PX], bf16, name="esm")
                wsm = big.tile([NENC, PX], bf16, name="wsm")
                r4 = big.tile([4, PX], bf16)

                def softmax_chunk(c):
                    sl = slice(c * CHUNK, (c + 1) * CHUNK)
                    # BN output is ~N(0,1): exp without max-subtraction is
                    # safe in f32.
                    nc.scalar.activation(
                        esm[:, sl], y2[:, sl], Act.Exp, bias=sc2[:, 1:2], scale=sc2[:, 0:1]
                    )
                    pd = ps.tile([P, CHUNK], fp32, tag="sm", bufs=2, name="pd")[:4, :CHUNK]
                    nc.tensor.matmul(pd[:], sel4[:], esm[:, sl], start=True, stop=True)
                    with nc.allow_low_precision("softmax denominators: bf16 ample for 2e-2 tolerance"):
                        nc.vector.reciprocal(r4[:, sl], pd[:])
                    pr = ps.tile([P, CHUNK], fp32, tag="sm", bufs=2, name="pr")[:NENC, :CHUNK]
                    nc.tensor.matmul(pr[:], sel100[:], r4[:, sl], start=True, stop=True)
                    nc.vector.tensor_tensor(wsm[:, sl], esm[:, sl], pr[:], Alu.mult)
                    # stage weights to DRAM for the DMA broadcasts
                    nc.sync.dma_start(t0_d[c][:], wsm[:, sl])

                # ---- reassembly ----
                def xview(k, r0, rep):
                    dy, dx = k // 5, k % 5
                    xv = xpad[:, r0 + dy : r0 + dy + 8, dx : dx + W]
                    return bass.AP(
                        xv.tensor, xv.offset, [xv.ap[0], [0, rep]] + list(xv.ap[1:])
                    )

                pool_taps_fixed = [k for k in range(25) if PROD[k] == "POOL"]

                def bcast_dma_for(c, k, pool):
                    wb = pool.tile([P, 4 * CHUNK], bf16, tag="wbd", name="wbd")
                    src = bass.AP(
                        t0_d[c].tensor,
                        t0_d[c].offset + 4 * k * CHUNK,
                        [[0, P], [1, 4 * CHUNK]],
                    )
                    nc.sync.dma_start(wb[:], src)
                    return wb

                def pool_product(c, k, wb):
                    tm = tmpP.tile([P, 4 * CHUNK], bf16, tag="tmP", name="tmP")
                    nc.gpsimd.tensor_tensor(tm[:], wb[:], xview(k, c * 8, 4), Alu.mult)
                    return tm

                def pool_chunk(c):
                    # emit all Pool-tap work for chunk c (runs one chunk ahead)
                    res = {}
                    wbs = [(k, bcast_dma_for(c, k, wbdPp)) for k in pool_taps_fixed]
                    for k, wb in wbs:
                        res[k] = pool_product(c, k, wb)
                    return res

                softmax_chunk(0)
                pool_tm = pool_chunk(0)
                softmax_chunk(1)

                for c in range(NCHUNK):
                    r0 = c * 8
                    col = slice(c * CHUNK, (c + 1) * CHUNK)
                    # 4 independent 1-bank accumulators: the next chunk can
                    # reuse bank s as soon as its stage copy drains it.
                    acc_ps = [
                        ps.tile([P, CHUNK], fp32, tag=f"acc{s}", bufs=1, name=f"acc{s}")
                        for s in range(4)
                    ]

                    # broadcast producers -------------------------------------
                    def bcast_cast(k):
                        wbS = wbsp.tile([P, 4 * CHUNK], bf16, tag="wbs", name="wbS")
                        for h in range(4):
                            wps = ps.tile(
                                [P, CHUNK], fp32, tag=("wps" if h % 2 == 0 else "sm"),
                                bufs=2, name="wps",
                            )
                            ch = 4 * k + h
                            onehot = eye100[:, ch : ch + 1].to_broadcast((NENC, P))
                            nc.tensor.matmul(
                                wps[:], onehot, wsm[:, col], start=True, stop=True
                            )
                            nc.scalar.activation(
                                wbS[:, h * CHUNK : (h + 1) * CHUNK], wps[:], Act.Copy
                            )
                        return wbS

                    def dve_product(k, wb):
                        tm = tmp.tile([P, 4 * CHUNK], bf16, tag="tm", name="tm")
                        nc.vector.tensor_tensor(tm[:], wb[:], xview(k, r0, 4), Alu.mult)
                        return tm

                    def acc(tm, first, last):
                        for s in range(4):
                            nc.tensor.matmul(
                                acc_ps[s][:], eye128[:],
                                tm[:, s * CHUNK : (s + 1) * CHUNK],
                                start=first, stop=last, skip_group_check=True,
                            )

                    # accumulation order: pool taps (precomputed last chunk)
                    # interleaved with DVE taps so acc can start immediately.
                    dve_taps = [k for k in range(25) if PROD[k] != "POOL"]
                    if c < 2:
                        # startup: CAST weights (PE+Act) are ready ~4us before
                        # the DRAM-staged DMA broadcasts; pool products land
                        # late (DMA-fed) so their accumulates go last.
                        cast_first = [k for k in dve_taps if BC[k] == "CAST"]
                        dma_rest = [k for k in dve_taps if BC[k] != "CAST"]
                        order = cast_first + dma_rest + list(pool_taps_fixed)
                    else:
                        # front-load 3 ready pool tiles, spread the rest
                        pool_pos = {0, 1, 8, 12, 16, 20}
                        order = []
                        li, pi = 0, 0
                        for i in range(25):
                            if pi < len(pool_taps_fixed) and i in pool_pos:
                                order.append(pool_taps_fixed[pi]); pi += 1
                            else:
                                order.append(dve_taps[li]); li += 1
                        order += pool_taps_fixed[pi:] + dve_taps[li:]

                    def prep(k):
                        if BC[k] == "CAST":
                            return bcast_cast(k)
                        return bcast_dma_for(c, k, wbdp)

                    merge_pts = set()
                    if N_MERGE > 0:
                        step = max(1, 25 // (N_MERGE + 1))
                        merge_pts = {step * (i + 1) for i in range(N_MERGE)}
                    dve_order = [k for k in order if PROD[k] != "POOL"]
                    pres = {}
                    AHEAD = 7
                    for i in range(min(AHEAD, len(dve_order))):
                        pres[dve_order[i]] = prep(dve_order[i])
                    npre = min(AHEAD, len(dve_order))
                    # next chunk's pool work interleaves with this chunk
                    nxt_pool = {}
                    pool_emit = {4: "wb", 8: "prod"} if False else None
                    nxt_wbs = []
                    pending = None
                    started = False
                    di = 0  # index into dve_order of next prep to emit
                    di = npre
                    for i in range(25):
                        k = order[i]
                        if PROD[k] == "POOL":
                            tm = pool_tm.pop(k)
                            # interleave next chunk's pool broadcasts/products
                            if c + 1 < NCHUNK:
                                j = pool_taps_fixed.index(k)
                                kj = pool_taps_fixed[j]
                                nxt_wbs.append((kj, bcast_dma_for(c + 1, kj, wbdPp)))
                        else:
                            if di < len(dve_order):
                                pres[dve_order[di]] = prep(dve_order[di])
                                di += 1
                            tm = dve_product(k, pres.pop(k))
                        if i in merge_pts and pending is not None:
                            tm2 = tmp.tile([P, 4 * CHUNK], bf16, tag="tm", name="tmm")
                            nc.vector.tensor_tensor(tm2[:], tm[:], pending[:], Alu.add)
                            pending = tm2
                        else:
                            if pending is not None:
                                acc(pending, first=not started, last=False)
                                started = True
                            pending = tm
                        # emit next chunk's pool products spread through chunk
                        if c + 1 < NCHUNK and nxt_wbs and i % 4 == 2:
                            kj, wbj = nxt_wbs.pop(0)
                            nxt_pool[kj] = pool_product(c + 1, kj, wbj)
                    acc(pending, first=not started, last=True)
                    if c + 1 < NCHUNK:
                        for kj, wbj in nxt_wbs:
                            nxt_pool[kj] = pool_product(c + 1, kj, wbj)
                        pool_tm = nxt_pool

                    # de-interleave (s=(di,dj), i, j) -> out rows (2i+di), cols (2j+dj)
                    # (emitted before the pipelined softmax so the stage copies
                    # sit early in the Act queue and release acc_ps quickly)
                    stage = stgp.tile([P, 4 * CHUNK], bf16, tag="stg", name="stage")
                    for s in range(4):
                        di_, dj_ = s // 2, s % 2
                        stg_v = bass.AP(
                            stage.tensor, stage.offset + di_ * 128 + dj_,
                            [stage.ap[0], [256, 8], [2, 64]],
                        )
                        acc_v = bass.AP(
                            acc_ps[s].tensor, acc_ps[s].offset,
                            [acc_ps[s].ap[0], [64, 8], [1, 64]],
                        )
                        nc.scalar.activation(stg_v, acc_v, Act.Copy)
                    nc.sync.dma_start(
                        out_d[:, c * 4 * CHUNK : (c + 1) * 4 * CHUNK], stage[:]
                    )

                    # pipeline the next softmax chunk while PE accumulates
                    if c + 2 < NCHUNK:
                        softmax_chunk(c + 2)

    nc.compile()
    return nc


def _prep_shared(comp_w, comp_g, comp_b, enc_w, enc_g, enc_b):
    import ml_dtypes

    bf = ml_dtypes.bfloat16
    w1t = np.ascontiguousarray(comp_w.reshape(MID, P).T).astype(bf)      # [128, 64]
    # w2t[tap] = enc_w[:, :, dy, dx].T  -> [64, 100] per tap, taps flattened
    w2t = np.ascontiguousarray(
        enc_w.transpose(2, 3, 1, 0).reshape(9, MID, NENC).transpose(1, 0, 2).reshape(MID, 9 * NENC)
    ).astype(bf)
    g1b1 = np.stack([comp_g, comp_b], axis=1).astype(np.float32)    # [64, 2]
    g2b2 = np.stack([enc_g, enc_b], axis=1).astype(np.float32)      # [100, 2]
    ch = np.arange(NENC)
    sel4 = (ch[:, None] % 4 == np.arange(4)[None, :]).astype(bf)   # [100, 4]
    sel100 = np.ascontiguousarray(sel4.T)                           # [4, 100]
    eye100 = np.eye(NENC, dtype=np.float32).astype(bf)
    eye128 = np.eye(P, dtype=np.float32).astype(bf)
    return dict(w1t=w1t, w2t=w2t, g1b1=g1b1, g2b2=g2b2, sel4=sel4, sel100=sel100, eye100=eye100, eye128=eye128)


def kernel(x, comp_w, comp_g, comp_b, enc_w, enc_g, enc_b):
    import ml_dtypes

    from concourse.bass_utils import run_bass_kernel_spmd

    x = np.asarray(x, np.float32)
    shared = _prep_shared(
        np.asarray(comp_w, np.float32), np.asarray(comp_g, np.float32),
        np.asarray(comp_b, np.float32), np.asarray(enc_w, np.float32),
        np.asarray(enc_g, np.float32), np.asarray(enc_b, np.float32),
    )
    if "nc" not in _CACHE:
        _CACHE["nc"] = _build_program()
    nc = _CACHE["nc"]

    in_maps = []
    for i in range(NB):
        m = dict(shared)
        m["x"] = np.ascontiguousarray(x[i].reshape(P, PX)).astype(ml_dtypes.bfloat16)
        in_maps.append(m)

    res = run_bass_kernel_spmd(nc, in_maps, list(range(NB)))
    out = np.stack([
        res.results[i]["out"].astype(np.float32).reshape(P, HM, HM) for i in range(NB)
    ])
    return out.astype(np.float32)
